# revision 6
# baseline (speedup 1.0000x reference)
"""Trainium2 Bass kernel for nn_CrossAttention (B=2, S=2048, E=1024, H=16, d=64).

Sharding: 8 cores = 2 batches x 4 query-blocks of 512 rows. Each core gets its
query block + full values[b]; no collectives.

Per-core pipeline (per head-pair p, heads A=2p, B=2p+1):
  1. kT = blkdiag(C*Wkv.T) @ vinT  (feature-major, C=16/ln2 folds the exp
     scale); qT = blkdiag(Wq.T) @ qinT + bq.
  2. scores.T psum tiles [128 kv, 1024] per 2-kv-tile group; head A rows 0:64
     of kT, head B rows 64:128, quadrant-packed via tile_position.
  3. exp split across engines: head A on ACT (true Exp, bf16 out), head B on
     DVE via the int16 bit-trick  E = bitcast_bf16(max(psum + B16, 0)).
  4. M = [v_nat | 1].T @ E  accumulated per head in one [65,1024] psum
     (associativity: U = Wv M, so the Wv projection collapses to step 5 and
     values are DMA'd in natural layout with a host-baked ones column).
  5. U2 = wv_ext @ M_sb -> [128,1024] psum = [U ; row-sum r replicated x64].
  6. o8 = U / r  (DVE divide, bf16) -> feature-major attention output.
  7. out = o8-slices @ WdT + bd  -> natural layout -> DMA.
"""

import sys

for _p in ("/opt/trn_rl_repo",):
    if _p not in sys.path:
        sys.path.insert(0, _p)

from contextlib import ExitStack

import ml_dtypes
import numpy as np

import concourse.bass as bass
import concourse.tile as tile
from concourse import bacc, mybir
from concourse.bass_utils import run_bass_kernel_spmd

F32 = mybir.dt.float32
BF16 = mybir.dt.bfloat16
I16 = mybir.dt.int16
EXP = mybir.ActivationFunctionType.Exp
ADD = mybir.AluOpType.add
MAX = mybir.AluOpType.max
DIV = mybir.AluOpType.divide

B, S, E, H, D = 2, 2048, 1024, 16, 64
N_CORES = 8
SQB = S * B // N_CORES  # 512 query rows per core
NP_BF16 = ml_dtypes.bfloat16

LN2 = float(np.log(2.0))
C_K = 0.125 * 128 / LN2          # kT scale so psum = (128/ln2) * scaled_score
SC_ACT = LN2 / 128.0             # ACT exp input scale
Z_ACT = -4.0                     # ACT exp zero point (cancels per-head)
# DVE bf16 bit-trick: i16 = floor(max(psum + B16, 0)); bitcast -> bf16
# B16 = 16256 + 128*Z/ln2 - 128*log2(1.0303) - 0.5  (PWL centering + floor)
B16 = float(16256 - 128 * 4 / LN2 - 128 * np.log2(1.0303) - 0.5)

_CACHE = {}


def _build_program():
    nc = bacc.Bacc("TRN2", target_bir_lowering=False, debug=False, num_devices=N_CORES)

    qT_in = nc.dram_tensor("qT_in", [E, SQB], BF16, kind="ExternalInput").ap()
    vT_in = nc.dram_tensor("vT_in", [E, S], BF16, kind="ExternalInput").ap()
    v5_in = nc.dram_tensor("v5_in", [H, 128, 16 * 65], BF16, kind="ExternalInput").ap()
    wkv2 = nc.dram_tensor("wkv2", [128, 128], BF16, kind="ExternalInput").ap()
    wq2 = nc.dram_tensor("wq2", [128, 128], BF16, kind="ExternalInput").ap()
    bq2 = nc.dram_tensor("bq2", [128, 1], F32, kind="ExternalInput").ap()
    wvx = nc.dram_tensor("wvx", [65, 128], BF16, kind="ExternalInput").ap()
    wdT = nc.dram_tensor("wdT", [E, E], BF16, kind="ExternalInput").ap()
    bd_rep = nc.dram_tensor("bd_rep", [128, E], F32, kind="ExternalInput").ap()
    out = nc.dram_tensor("out", [SQB, E], F32, kind="ExternalOutput").ap()

    with tile.TileContext(nc) as tc, ExitStack() as ctx:
        wpool = ctx.enter_context(tc.tile_pool(name="w", bufs=1))
        vtp = ctx.enter_context(tc.tile_pool(name="vtp", bufs=3))
        qinp = ctx.enter_context(tc.tile_pool(name="qinp", bufs=2))
        v5p = ctx.enter_context(tc.tile_pool(name="v5p", bufs=2))
        ktp = ctx.enter_context(tc.tile_pool(name="ktp", bufs=2))
        qtp = ctx.enter_context(tc.tile_pool(name="qtp", bufs=2))
        ep = ctx.enter_context(tc.tile_pool(name="ep", bufs=3))
        msbp = ctx.enter_context(tc.tile_pool(name="msb", bufs=2))
        recp = ctx.enter_context(tc.tile_pool(name="rec", bufs=2))
        o8p = ctx.enter_context(tc.tile_pool(name="o8", bufs=1))
        osbp = ctx.enter_context(tc.tile_pool(name="osb", bufs=2))
        sc_ps = ctx.enter_context(tc.tile_pool(name="scps", bufs=2, space="PSUM"))
        m_ps = ctx.enter_context(tc.tile_pool(name="mps", bufs=1, space="PSUM"))
        u_ps = ctx.enter_context(tc.tile_pool(name="ups", bufs=1, space="PSUM"))

        # ---- constants ----
        wkv2_s = wpool.tile([128, 128], BF16, tag="wkv2")
        nc.sync.dma_start(wkv2_s[:], wkv2[:])
        wq2_s = wpool.tile([128, 128], BF16, tag="wq2")
        nc.sync.dma_start(wq2_s[:], wq2[:])
        bq2_s = wpool.tile([128, 1], F32, tag="bq2")
        nc.sync.dma_start(bq2_s[:], bq2[:])
        wvx_s = wpool.tile([65, 128], BF16, tag="wvx")
        nc.sync.dma_start(wvx_s[:], wvx[:])
        bd_s = wpool.tile([128, E], F32, tag="bd")
        nc.sync.dma_start(bd_s[:], bd_rep[:])
        wd_s = []
        for kk in range(8):
            t = wpool.tile([128, E], BF16, tag=f"wd{kk}")
            nc.sync.dma_start(t[:], wdT[kk * 128 : (kk + 1) * 128, :])
            wd_s.append(t)
        zk_s = wpool.tile([128, 1], F32, tag="zk")
        nc.gpsimd.memset(zk_s[:], Z_ACT)

        o8 = [
            o8p.tile([128, 2, SQB], BF16, tag=f"o8_{J}", name=f"o8_{J}")
            for J in range(4)
        ]

        for p in range(8):
            # ---- input DMAs for this pair ----
            vt = vtp.tile([128, S], BF16, tag="vt", name=f"vt{p}")
            nc.sync.dma_start(vt[:], vT_in[p * 128 : (p + 1) * 128, :])
            qin = qinp.tile([128, SQB], BF16, tag="qin", name=f"qin{p}")
            nc.sync.dma_start(qin[:], qT_in[p * 128 : (p + 1) * 128, :])
            v5A = v5p.tile([128, 16, 65], BF16, tag="v5A", name=f"v5A{p}")
            nc.sync.dma_start(v5A[:], v5_in[2 * p])
            v5B = v5p.tile([128, 16, 65], BF16, tag="v5B", name=f"v5B{p}")
            nc.sync.dma_start(v5B[:], v5_in[2 * p + 1])

            # ---- projections ----
            kt = ktp.tile([128, S], BF16, tag="kt", name=f"kt{p}")
            for c in range(2):
                pk = sc_ps.tile([128, 1024], F32, tag="sc")
                nc.tensor.matmul(
                    pk[:, 0:512], wkv2_s[:], vt[:, c * 1024 : c * 1024 + 512],
                    start=True, stop=True,
                )
                nc.tensor.matmul(
                    pk[:, 512:1024], wkv2_s[:],
                    vt[:, c * 1024 + 512 : (c + 1) * 1024],
                    start=True, stop=True,
                )
                nc.scalar.copy(kt[:, c * 1024 : (c + 1) * 1024], pk[:])
            pq = sc_ps.tile([128, 1024], F32, tag="sc")
            nc.tensor.matmul(pq[:, 0:512], wq2_s[:], qin[:], start=True, stop=True)
            qt = qtp.tile([128, SQB], BF16, tag="qt", name=f"qt{p}")
            nc.vector.tensor_scalar_add(qt[:], pq[:, 0:512], bq2_s[:])

            # ---- scores -> exp -> M, streamed per 2-kv-tile group g ----
            MA = m_ps.tile([65, 1024], F32, tag="m")  # A cols 0:512, B 512:1024
            for g in range(8):
                psA = sc_ps.tile([128, 1024], F32, tag="sc")
                psB = sc_ps.tile([128, 1024], F32, tag="sc")
                for tt in range(2):
                    t = 2 * g + tt
                    nc.tensor.matmul(
                        psA[:, tt * 512 : (tt + 1) * 512],
                        kt[0:64, t * 128 : (t + 1) * 128], qt[0:64, :],
                        start=True, stop=True, tile_position=(0, 0),
                    )
                    nc.tensor.matmul(
                        psB[:, tt * 512 : (tt + 1) * 512],
                        kt[64:128, t * 128 : (t + 1) * 128], qt[64:128, :],
                        start=True, stop=True, tile_position=(64, 0),
                    )
                ea = ep.tile([128, 1024], BF16, tag="EA", name=f"ea{p}_{g}")
                nc.scalar.activation(ea[:], psA[:], EXP, scale=SC_ACT, bias=zk_s[:])
                eb = ep.tile([128, 1024], BF16, tag="EB", name=f"eb{p}_{g}")
                with nc.allow_low_precision(reason="bf16 bit-trick exp"):
                    nc.vector.tensor_scalar(
                        eb[:].bitcast(I16), psB[:], B16, 0.0, ADD, MAX,
                    )
                for tt in range(2):
                    t = 2 * g + tt
                    nc.tensor.matmul(
                        MA[:, 0:512], v5A[:, t, :],
                        ea[:, tt * 512 : (tt + 1) * 512],
                        start=(t == 0), stop=(t == 15),
                    )
                    nc.tensor.matmul(
                        MA[:, 512:1024], v5B[:, t, :],
                        eb[:, tt * 512 : (tt + 1) * 512],
                        start=(t == 0), stop=(t == 15),
                    )

            # ---- U2 = wv_ext @ M ; normalize ----
            msb = msbp.tile([65, 1024], BF16, tag="msb", name=f"msb{p}")
            nc.scalar.copy(msb[:], MA[:])
            u2 = u_ps.tile([128, 1024], F32, tag="u")
            nc.tensor.matmul(u2[:, 0:512], wvx_s[:], msb[:, 0:512],
                             start=True, stop=True)
            nc.tensor.matmul(u2[:, 512:1024], wvx_s[:], msb[:, 512:1024],
                             start=True, stop=True)
            J, sp = p // 2, p % 2
            rec = recp.tile([64, 1024], BF16, tag="rec", name=f"rec{p}")
            with nc.allow_low_precision(reason="bf16 softmax denom"):
                nc.vector.reciprocal(rec[:], u2[64:128, :])
            with nc.allow_low_precision(reason="bf16 attention output"):
                nc.vector.tensor_mul(
                    o8[J][0:64, sp, :], u2[0:64, 0:512], rec[:, 0:512],
                )
                nc.vector.tensor_mul(
                    o8[J][64:128, sp, :], u2[0:64, 512:1024], rec[:, 512:1024],
                )

        # ---- output projection + bias -> natural layout -> DMA ----
        for m in range(4):
            po = sc_ps.tile([128, 1024], F32, tag="sc")
            for n in range(2):
                for kk in range(8):
                    J, sp = kk // 2, kk % 2
                    nc.tensor.matmul(
                        po[:, n * 512 : (n + 1) * 512],
                        o8[J][:, sp, m * 128 : (m + 1) * 128],
                        wd_s[kk][:, n * 512 : (n + 1) * 512],
                        start=(kk == 0), stop=(kk == 7),
                    )
            ob = osbp.tile([128, E], F32, tag="ob")
            nc.vector.tensor_add(ob[:], po[:], bd_s[:])
            nc.sync.dma_start(out[m * 128 : (m + 1) * 128, :], ob[:])

    nc.compile()
    return nc


def kernel(queries, values, heads, Wv, bv, Wk, bk, Wq, bq, Wd, bd, **_):
    queries = np.asarray(queries, np.float32)
    values = np.asarray(values, np.float32)
    Wv, bv = np.asarray(Wv, np.float32), np.asarray(bv, np.float32)
    Wk = np.asarray(Wk, np.float32)
    Wq, bq = np.asarray(Wq, np.float32), np.asarray(bq, np.float32)
    Wd, bd = np.asarray(Wd, np.float32), np.asarray(bd, np.float32)
    assert int(heads) == H and queries.shape == (B, S, E)

    if "nc" not in _CACHE:
        _CACHE["nc"] = _build_program()
    nc = _CACHE["nc"]

    def blk(A):
        Z = np.zeros_like(A)
        return np.block([[A, Z], [Z, A]]).astype(NP_BF16)

    Wkv = Wk @ Wv
    wkv2 = blk((C_K * Wkv).T)
    wq2 = blk(Wq.T)
    bq2 = np.concatenate([bq, bq])[:, None].astype(np.float32)
    # wvx: [65,128]; cols 0:64 -> Wv.T rows (U = Wv @ M), col 64 unused for
    # m<64; cols 64:128 replicate the denominator row (M row 64).
    wvx = np.zeros((65, 128), np.float32)
    wvx[0:64, 0:64] = Wv.T
    wvx[64, 64:128] = 1.0
    bv_full = np.tile(bv, H)
    bd_rep = np.tile((bd + Wd @ bv_full)[None, :], (128, 1)).astype(np.float32)
    wdT = np.ascontiguousarray(Wd.T).astype(NP_BF16)

    # v5: [H, 128, 16*65] natural values + ones column, per batch
    v5_b = []
    for b_ in range(B):
        v4 = values[b_].reshape(16, 128, H, D).transpose(2, 1, 0, 3)  # h,p,t,d
        v5 = np.empty((H, 128, 16, 65), np.float32)
        v5[..., 0:64] = v4
        v5[..., 64] = 1.0
        v5_b.append(np.ascontiguousarray(v5.reshape(H, 128, 16 * 65)).astype(NP_BF16))
    vT_b = [
        np.ascontiguousarray(values[b_].T).astype(NP_BF16) for b_ in range(B)
    ]

    common = dict(wkv2=wkv2, wq2=wq2, bq2=bq2, wvx=wvx.astype(NP_BF16),
                  wdT=wdT, bd_rep=bd_rep)
    in_maps = []
    for c in range(N_CORES):
        b_, qb = c // 4, c % 4
        in_maps.append(dict(
            qT_in=np.ascontiguousarray(
                queries[b_, qb * SQB : (qb + 1) * SQB, :].T
            ).astype(NP_BF16),
            vT_in=vT_b[b_],
            v5_in=v5_b[b_],
            **common,
        ))

    _CACHE["last_in_maps"] = in_maps
    res = run_bass_kernel_spmd(nc, in_maps, list(range(N_CORES)))
    out = np.empty((B, S, E), np.float32)
    for c in range(N_CORES):
        b_, qb = c // 4, c % 4
        out[b_, qb * SQB : (qb + 1) * SQB, :] = res.results[c]["out"]
    return out
